# revision 94
# baseline (speedup 1.0000x reference)
"""Cross-level attention (3 KV levels: causal T=2048, full T1=512, full T2=128)
for B=2, H=16, T=2048, DH=64 on 8 Trainium2 NeuronCores.

Sharding: the 32 (b, h) pairs are split 4-per-core (batch + head parallel);
each core computes full attention for its 4 heads, level_w replicated.

Per-core dataflow (all operands resident in SBUF after one initial load):
  - Heads are processed as 2 "duos" (2 pairs packed on partitions 0-63 /
    64-127) so the two K=64-contraction QK^T matmuls occupy disjoint PE row
    groups and run concurrently.
  - S^T tiles [s=128, t=512] are computed per 128-wide K-block j via
    matmul(lhsT=K^T, rhs=Q^T).  The softmax exp is SPLIT ACROSS TWO ENGINES:
    ~70% of blocks run on the Scalar engine (activation Exp out of PSUM into
    bf16 P^T, 1 elem/lane/cycle), and ~27 full L1/L2 blocks per core run on
    the Vector engine via two registered custom-DVE ops: ANT_CLA_EXP_P1
    (deg-4 Horner ~= exp(u), 8 ALU stages, b0=b1=1 so 3 scalar slots
    suffice) then ANT_CLA_EXP_P2 (y^16 via 4 squarings, one instruction per
    PAIR of blocks).  Q is host-pre-scaled by 1/128 = (1/sqrt(dh))*(1/16) so
    DVE gets poly-range u directly; ACT compensates with scale=16.
  - Causal masking for level 0 is block-level: fully-masked blocks are
    skipped, diagonal blocks compute only [f0,512) and get a 0/1 upper-tri
    multiply post-exp on the gpsimd engine.
  - PV: matmul(lhsT=P^T[:, c-slice], rhs=V'[s-block]) accumulating in PSUM,
    V' carrying a ones-column so the denominator lands in column 64 free.
    Level weights are folded into V on the host.  Combine = reciprocal +
    per-column broadcast multiply (DVE) + level adds (gpsimd), per (half,
    level) so only one accumulator bank is live per group.
  - PSUM (8 banks): qkp 2x[128,2,512] double-buffered ACT score tiles (4) +
    qkd 1x[128,2,512] DVE score tile (2; P1 drains it while PE refills
    behind the in-order DVE queue) + pvp 2 rotating PV accumulator banks.
    Separate qkp/qkd pools keep the two exp streams decoupled — a shared
    rotation serializes ACT behind the 2.3x-slower DVE drain.
  - Emission is an interleaved slot schedule: [first 2 ACT blocks (runway)]
    then ACT blocks / DVE pairs / previous unit's PV-group+combine chunks
    merged at hand-tuned slot fractions, so each in-order engine queue sees
    its work in dependency-ready order.  The final slot weaves the last two
    units' PV chunks into one bank-rotation-consistent sequence to shrink
    the drain tail.  DMA: HWDGE ops cost 625ns each, so K1+K2 and the three
    V levels are merged into single host-side tensors, qt/k0t stream in
    512-col chunks in unit-consumption order, K12/V/tri ride the parallel
    gpsimd SWDGE queue, and outputs batch both halves per unit.

TimelineSim cost model: 107.8 us/core (was 130 us on the all-ACT baseline).
Busy: ACT 87.6 us (69 us exp elements + call overheads), DVE 87.2 us (63 us
custom-exp + 24 us combine/recip), PE 69.8 us, Pool 46 us.  HW-validated:
rel-l2 2.8e-3 (the deg-4+^16 poly adds 1.4e-4, invisible under bf16).
Late wins: k0t chunk ([0:128] then rest) ahead of qt in the HWDGE queue so
the first Ldweights overlaps the qt transfer (-1.1 us); the final unit's
last level-add on DVE instead of the Pool q7-launch path (-0.2 us).

Notes for future iteration: custom-DVE exp is priced at 1 elem/lane/cycle
(no 2x for fused specs), so the ACT/DVE marginal ratio is ~2.3:1 and the
routing (~27 blocks) balances the two engines at ~87 us; remaining slack is
~6 us DMA-gated startup, ~5 us mid-run combine/recip waits in heavy-PV
slots, ~6 us drain.  Measured dead ends: sharing one 3-buf qkp pool for
both exp streams (+6 us), full-unit P2 batching (+2 us), L1/L2 combine
muls on gpsimd (+2 us), routing L1+L2 of the DMA-gated first units to DVE
(+3 us), SWDGE for qt chunks (+6 us: 994ns/op on the Pool engine and it
queues ahead of the V loads), splitting heavy L0 PV-group emission (+0.4 us:
delays the stop the reciprocal waits on), PE p-state warm-up matmuls
(neutral: the model's ramp is wall-clock based and already warm at 3.2 us),
issuing the first DMAs from the ACT/DVE HWDGE queues (neutral: startup is
fixed-latency dominated).
"""
import numpy as np
import ml_dtypes

B, H, T, DH = 2, 16, 2048, 64
C = H * DH
T1, T2 = 512, 128
NCORES = 8
PAIRS = 4          # (b, h) pairs per core
DUOS = 2           # pairs are packed two-per-SBUF-tile
QB = T // 512      # 512-wide query blocks
NJ = (T // 128, T1 // 128, T2 // 128)

TRACE = False          # set by test.py for profiling runs
LAST_RESULT = None     # BassKernelResults from the most recent run

_NC_CACHE = {}

# Custom-DVE exp: P1 = deg-4 Horner ~= exp(u) on u in [-0.385, 0.385] with
# b0=b1=1 enforced (3 free coeffs fit C0/C1/imm2); P2 = 4 squarings -> exp(16u).
# Q is pre-scaled by 1/(8*16) on the host so PSUM scores arrive as u = s/128;
# the ACT path compensates with activation scale=16. Poly rel err 9e-6 (x16 ->
# 1.4e-4), well under bf16 output rounding. HW-validated bit-exact vs numpy.
_EXP_B2 = 0.5000775124599794
_EXP_B3 = 0.1677372765728442
_EXP_B4 = 0.04118887451149361
_EXP_OPS = {}


def _register_exp_ops():
    if _EXP_OPS:
        return _EXP_OPS
    import concourse.dve_ops as dve_ops
    from concourse.dve_ops import DveOp
    from concourse.dve_spec import Spec, Src0, C0, C1, C2, One, sq, lower
    from concourse.dve_uop import DveOpSpec

    def _ref_p1(in0, in1, s0, s1, imm2):
        return (((s0 * in0 + s1) * in0 + imm2) * in0 + 1.0) * in0 + 1.0

    def _ref_p2(in0, in1, s0, s1, imm2):
        y = in0 * in0
        y = y * y
        y = y * y
        return y * y

    _h = Src0 * C0 + C1
    _h = _h * Src0 + C2
    _h = _h * Src0 + One
    specs = {
        "ANT_CLA_EXP_P1": Spec(body=_h * Src0 + One, reference=_ref_p1),
        "ANT_CLA_EXP_P2": Spec(body=sq(sq(sq(sq(Src0)))), reference=_ref_p2),
    }
    for name, spec in specs.items():
        existing = next((o for o in dve_ops.OPS if o.name == name), None)
        if existing is not None:
            _EXP_OPS[name] = existing
            continue
        row = dve_ops._CUSTOM_DVE_ROW_BASE + len(dve_ops.OPS)
        assert row < 0x20
        dve_ops._SUB_OPCODE_FOR_NAME[name] = row
        shas = {}
        for ver in ("v3", "v4"):
            s = DveOpSpec(name=name, opcode=row, uops=lower(spec, ver=ver),
                          rd1_en=False)
            shas[ver] = s.sha(ver)
        op = DveOp(name, spec, subdim=False, uops_sha=shas)
        dve_ops.OPS.append(op)
        dve_ops.CUSTOM_DVE_SPECS[name] = spec
        _EXP_OPS[name] = op
    return _EXP_OPS


def _build_nc(w):
    import concourse.bass as bass
    from concourse import bacc
    import concourse.tile as tile
    import concourse.mybir as mybir
    from contextlib import ExitStack

    BF16 = mybir.dt.bfloat16
    F32 = mybir.dt.float32
    Exp = mybir.ActivationFunctionType.Exp
    ops = _register_exp_ops()
    EXP_P1 = ops["ANT_CLA_EXP_P1"]
    EXP_P2 = ops["ANT_CLA_EXP_P2"]

    nc = bacc.Bacc("TRN2", target_bir_lowering=False)
    qt = nc.dram_tensor("qt", [DUOS, 128, T], BF16, kind="ExternalInput")
    k0t = nc.dram_tensor("k0t", [DUOS, 128, T], BF16, kind="ExternalInput")
    k12t = nc.dram_tensor("k12t", [DUOS, 128, T1 + T2], BF16,
                          kind="ExternalInput")
    vv = nc.dram_tensor("vv", [PAIRS, 128, NJ[0] + NJ[1] + NJ[2], 65], BF16,
                        kind="ExternalInput")
    tri = nc.dram_tensor("tri", [128, 2, 128], BF16, kind="ExternalInput")
    out = nc.dram_tensor("out", [DUOS, 128, 2, T // 128, DH], F32,
                         kind="ExternalOutput")

    with tile.TileContext(nc) as tc, ExitStack() as ctx:
        # PSUM budget (8 banks): qkp 2x[128,2,512] (4) + qkd 1x[128,2,512] (2)
        # + pvp 2x[128,4,65] (2).
        const = ctx.enter_context(tc.tile_pool(name="const", bufs=1))
        qkp = ctx.enter_context(tc.tile_pool(name="qkp", bufs=2, space="PSUM"))
        qkd = ctx.enter_context(tc.tile_pool(name="qkd", bufs=1, space="PSUM"))
        pvp = ctx.enter_context(tc.tile_pool(name="pvp", bufs=2, space="PSUM"))
        pts = ctx.enter_context(tc.tile_pool(name="pts", bufs=40))
        ptd = ctx.enter_context(tc.tile_pool(name="ptd", bufs=8))
        expf = ctx.enter_context(tc.tile_pool(name="expf", bufs=4))
        outp = ctx.enter_context(tc.tile_pool(name="outp", bufs=10))
        small = ctx.enter_context(tc.tile_pool(name="small", bufs=12))

        def load(dram_ap, shape, tag):
            t = const.tile(shape, BF16, tag=tag, name=tag)
            nc.gpsimd.dma_start(out=t, in_=dram_ap)
            return t

        # Big Q/K tensors are DMA'd in 512-column chunks so the first
        # query-block's matmuls can start before the full tensors land.
        def load_chunked(dram_ap, shape, tag, chunk=512):
            t = const.tile(shape, BF16, tag=tag, name=tag)
            for c0 in range(0, shape[1], chunk):
                nc.sync.dma_start(out=t[:, c0:c0 + chunk],
                                  in_=dram_ap[:, c0:c0 + chunk])
            return t

        # Explicit zero-bias tile for the ACT exp calls, memset on the (idle
        # at t=0) DVE: the implicit float-bias path materializes a const AP
        # whose backing DMA would gate the first exp behind the HWDGE queue.
        zbias = const.tile([128, 1], F32, tag="zbias", name="zbias")
        nc.vector.memset(zbias, 0.0)

        # Emission order = need order: the first unit is (d=0, qb=0), which
        # needs only qt0/k0t0 column chunk 0 plus k1t/k2t of duo 0. V tiles go
        # on the gpsimd SWDGE queue so they stream in parallel with the
        # HWDGE-queued Q/K chunks.
        sb_tri = const.tile([128, 2, 128], BF16, tag="tri", name="tri")
        def alloc(shape, tag):
            return const.tile(shape, BF16, tag=tag, name=tag)
        sb_qt = [alloc([128, T], f"qt{d}") for d in range(DUOS)]
        sb_k12 = [alloc([128, T1 + T2], f"k12t{d}") for d in range(DUOS)]
        sb_kt = [[alloc([128, T], f"k0t{d}") for d in range(DUOS)],
                 [t[:, 0:T1] for t in sb_k12],
                 [t[:, T1:T1 + T2] for t in sb_k12]]
        sb_vv = [alloc([128, NJ[0] + NJ[1] + NJ[2], 65], f"vv{p}")
                 for p in range(PAIRS)]
        sb_v = [[t[:, 0:NJ[0]] for t in sb_vv],
                [t[:, NJ[0]:NJ[0] + NJ[1]] for t in sb_vv],
                [t[:, NJ[0] + NJ[1]:] for t in sb_vv]]

        def dma_cols(t, dram_ap, c0, c1, engine=None):
            (engine or nc.sync).dma_start(out=t[:, c0:c1], in_=dram_ap[:, c0:c1])

        # DMA emission matches unit consumption order: duo 0 runs qb
        # ascending (each new qb needs the next qt chunk and one more k0t
        # chunk), duo 1 runs qb descending (needs all of k0t and the qt tail
        # first).
        for d in range(DUOS):
            # unit (d, qb0): first 512 cols of qt/k0t + all of k1t/k2t.
            # k0t's first chunk is split so j=0's 128 columns land first and
            # the very first QK matmul can issue as early as possible.
            # qt chunks ride the gpsimd SWDGE queue: it runs in parallel with
            # the (625ns/op serial) HWDGE, so K and Q stream concurrently and
            # the first QK matmul isn't gated on a single DGE pipe.
            dma_cols(sb_kt[0][d], k0t[d], 0, 128)
            if d == 0:
                dma_cols(sb_qt[d], qt[d], 0, 512)
            dma_cols(sb_kt[0][d], k0t[d], 128, 512)
            # merged K1+K2 on the SWDGE queue (parallel with HWDGE; the
            # DVE-routed L1/L2 blocks start a bit later anyway)
            nc.gpsimd.dma_start(out=sb_k12[d], in_=k12t[d])
            if d == 0:
                nc.gpsimd.dma_start(out=sb_tri, in_=tri[:])
            for p in (2 * d, 2 * d + 1):
                nc.gpsimd.dma_start(out=sb_vv[p], in_=vv[p])
            if d == 0:
                # unit order (0,0),(0,1),(0,3),(0,2): after the shared first
                # chunks, (0,1) needs qt[512:1024]+k0t[512:1024], then (0,3)
                # needs the k0t tail + qt[1536:2048], then (0,2) the rest.
                dma_cols(sb_qt[d], qt[d], 512, 1024)
                dma_cols(sb_kt[0][d], k0t[d], 512, 1024)
                dma_cols(sb_kt[0][d], k0t[d], 1024, 1536)
                dma_cols(sb_kt[0][d], k0t[d], 1536, 2048)
                dma_cols(sb_qt[d], qt[d], 1536, 2048)
                dma_cols(sb_qt[d], qt[d], 1024, 1536)
            else:
                # qb descending: (1,3) needs all of k0t and qt[1536:2048].
                for c0 in range(512, T, 512):
                    dma_cols(sb_kt[0][d], k0t[d], c0, c0 + 512)
                for c0 in range(T - 512, -1, -512):
                    dma_cols(sb_qt[d], qt[d], c0, c0 + 512)

        def unit_jblocks(qb):
            return ([(0, j) for j in range(4 * qb + 4)]
                    + [(1, j) for j in range(NJ[1])]
                    + [(2, j) for j in range(NJ[2])])

        def route_to_dve(l, j, qb, u):
            # Full (non-diag) blocks offloaded to the DVE custom-exp path,
            # sized to balance ACT (~0.98us/block) vs DVE (~2.4us/block +
            # retained combine work).  The last unit keeps everything on ACT
            # so the drain tail isn't gated on the DVE exp stream.
            if u == (1, 0):
                return False
            if l == 1 and j >= 1:
                return True
            if l == 2 and qb >= 1:
                return True
            return False

        def qk_matmuls(d, qb, l, j, f0, sp):
            for half in range(2):
                nc.tensor.matmul(
                    out=sp[:, half, f0:],
                    lhsT=sb_kt[l][d][64 * half:64 * half + 64,
                                     128 * j:128 * j + 128],
                    rhs=sb_qt[d][64 * half:64 * half + 64,
                                 512 * qb + f0:512 * qb + 512],
                    start=True, stop=True,
                )

        def emit_act_block(d, qb, l, j, pt):
            # Diagonal L0 blocks: columns left of the diagonal 128-col
            # sub-block are fully causal-masked and never read by PV,
            # so compute/exp only the [f0, 512) column range.
            f0 = 128 * (j - 4 * qb) if (l == 0 and j >= 4 * qb) else 0
            sp = qkp.tile([128, 2, 512], F32, tag="qk", name="qk")
            qk_matmuls(d, qb, l, j, f0, sp)
            nc.scalar.activation(out=pt[:, :, f0:], in_=sp[:, :, f0:],
                                 func=Exp, scale=16.0, bias=zbias[:, 0:1])
            if l == 0 and j >= 4 * qb:
                s = pt[:, :, f0:f0 + 128]
                nc.gpsimd.tensor_mul(out=s, in0=s, in1=sb_tri)

        def emit_dve_pair(d, qb, blocks, pt_tiles):
            # One or two DVE-routed blocks.  For a pair, P1 stays per-block
            # (PSUM-limited) but writes halves of one fp32 scratch tile, and
            # a single P2 (y^16, SBUF->SBUF) covers both blocks — halving the
            # P2 instruction count.
            n2 = 2 * len(blocks)
            yf = expf.tile([128, n2, 512], F32, tag="yf", name="yf")
            for k, (l, j) in enumerate(blocks):
                sp = qkd.tile([128, 2, 512], F32, tag="qkd", name="qkd")
                qk_matmuls(d, qb, l, j, 0, sp)
                nc.vector._custom_dve(EXP_P1, out=yf[:, 2 * k:2 * k + 2],
                                      in0=sp,
                                      s0=_EXP_B4, s1=_EXP_B3, imm2=_EXP_B2)
            pt = ptd.tile([128, n2, 512], BF16, tag="ptd", name="ptd")
            nc.vector._custom_dve(EXP_P2, out=pt, in0=yf)
            for k, (l, j) in enumerate(blocks):
                pt_tiles[(l, j)] = pt[:, 2 * k:2 * k + 2]

        def phase2_chunks(d, qb, pt_tiles):
            # ---- Phase 2 as a list of (rel_key, level, thunk) chunks: PV
            # accumulation groups + combines, interleaved into the next
            # slot's emission at rel_key (fraction of the slot).  pvp's 2
            # bufs rotate A,B,A,B,... across the 6 groups; each combine's
            # reads free its bank just before the next group needs it, and
            # combines are keyed ~1/3 slot after their PV group so the PE has
            # drained the group before the DVE reaches the reciprocal.
            jblocks = unit_jblocks(qb)
            state = {}

            def pv_group(half, l, cs=(0, 1, 2, 3)):
                def thunk():
                    p = 2 * d + half
                    if cs[0] == 0:
                        acc = pvp.tile([128, 4, 65], F32, tag="pv",
                                       name=f"pv{l}")
                        state[(half, l)] = acc
                    else:
                        acc = state[(half, l)]
                    jl = [j for (ll, j) in jblocks if ll == l]
                    lvl_last_j = jl[-1]
                    # One accumulation group per PSUM bank: start zeroes the
                    # whole 2KB zero region, so only the very first matmul
                    # into the bank may set start=True.
                    for j in jl:
                        pt = pt_tiles[(l, j)]
                        for c in cs:
                            if l == 0 and j > 4 * qb + c:
                                continue
                            nc.tensor.matmul(
                                out=acc[:, c, :],
                                lhsT=pt[:, half, 128 * c:128 * c + 128],
                                rhs=sb_v[l][p][:, j, :],
                                start=(j == 0 and c == 0),
                                stop=(j == lvl_last_j and c == 3),
                            )
                return thunk

            def combine(half, l, last, tail=False):
                def thunk():
                    acc = state.pop((half, l))
                    if ("osb",) not in state:
                        state[("osb",)] = outp.tile(
                            [128, 2, 4, DH], F32, tag="osb", name="osb")
                    osb = state[("osb",)][:, half]
                    rc = small.tile([128, 4, 1], F32, tag="rc", name="rc")
                    nc.vector.reciprocal(out=rc[:, :, 0], in_=acc[:, :, 64])
                    dst = osb if l == 0 else outp.tile([128, 4, DH], F32,
                                                       tag="tmp", name="tmp")
                    eng = nc.vector
                    eng.tensor_mul(
                        out=dst, in0=acc[:, :, 0:64],
                        in1=rc.broadcast_to([128, 4, DH]))
                    if l > 0:
                        ((nc.vector if (tail and half == 1 and l == 2)
                          else nc.gpsimd)
                         .tensor_add(out=osb, in0=osb, in1=dst))
                    if last and tail:
                        # final unit: per-half DMA so h0's write overlaps
                        # h1's combine
                        osb_full = state[("osb",)]
                        nc.sync.dma_start(
                            out=out[d][:, half:half + 1, 4 * qb:4 * qb + 4, :],
                            in_=osb_full[:, half:half + 1])
                        if half == 1:
                            state.pop(("osb",))
                    elif last and half == 1:
                        # one batched out-DMA per unit (both halves)
                        nc.sync.dma_start(
                            out=out[d][:, :, 4 * qb:4 * qb + 4, :],
                            in_=state.pop(("osb",)))
                return thunk

            return {"pv": pv_group, "cmb": combine}

        def std_p2_keys(p2, cshift=0.0, qb=0):
            # standard chunk schedule for one previous unit inside a slot.
            # Heavy units (qb>=2) split the big (h0,L0) PV group around one
            # act block so ACT gets an interleave point mid-group.
            c = min(cshift, 0.03)
            head = [(0.02, "pv", p2["pv"](0, 0)),
                    (0.08, "pv", p2["pv"](0, 1))]
            return head + [
                (0.36 + cshift, "cmb", p2["cmb"](0, 0, False)),
                (0.40 + cshift, "pv", p2["pv"](0, 2)),
                (0.48 + cshift, "cmb", p2["cmb"](0, 1, False)),
                (0.52 + c, "pv", p2["pv"](1, 0)),
                (0.60 + cshift, "cmb", p2["cmb"](0, 2, True)),
                (0.64 + c, "pv", p2["pv"](1, 1)),
                (0.76 + cshift, "cmb", p2["cmb"](1, 0, False)),
                (0.80 + c, "pv", p2["pv"](1, 2)),
                (0.87 + c, "cmb", p2["cmb"](1, 1, False)),
                (0.93 + c, "cmb", p2["cmb"](1, 2, True)),
            ]

        def emit_slot(u, p2c, pt_tiles=None):
            # One steady-state slot: ACT-path blocks of unit u carry the
            # schedule; the previous unit's phase2 chunks and u's DVE-path
            # blocks are interleaved between them so every engine's in-order
            # queue sees work in dependency-ready order (ACT runway first,
            # PV groups early, combines after their PV, DVE exp spread out).
            d, qb = u
            act_blocks, dve_blocks = [], []
            if pt_tiles is None:
                pt_tiles = {}
            for (l, j) in unit_jblocks(qb):
                if route_to_dve(l, j, qb, u):
                    dve_blocks.append((l, j))
                    continue
                act_blocks.append((l, j))
                if (l, j) not in pt_tiles:
                    pt_tiles[(l, j)] = pts.tile([128, 2, 512], BF16,
                                                tag="pt", name="pt")
            pairs = [tuple(dve_blocks[i:i + 2])
                     for i in range(0, len(dve_blocks), 2)]
            A, D = len(act_blocks), len(pairs)
            items = [((0.0 if i == 0 else 0.01 if i == 1
                       else (i + 1) / (A + 1)), 0, ("act",) + b)
                     for i, b in enumerate(act_blocks)]
            items += [((k + 0.8) / (D + 1), 1, ("dvp", pr))
                      for k, pr in enumerate(pairs)]
            items += [(key, 2, ("p2", thunk)) for (key, _, thunk) in p2c]
            items.sort(key=lambda x: (x[0], x[1]))
            for _, _, it in items:
                if it[0] == "act":
                    emit_act_block(d, qb, it[1], it[2], pt_tiles[it[1:]])
                elif it[0] == "dvp":
                    emit_dve_pair(d, qb, it[1], pt_tiles)
                else:
                    it[1]()
            return pt_tiles

        # Unit order: duo 0 ascending then duo 1 descending, matching the
        # DMA stream so early units are never DMA-gated; the last unit (1,0)
        # is the smallest (shortest drain tail).
        units = [(0, 0), (0, 1), (0, 3), (0, 2), (1, 3), (1, 2), (1, 1), (1, 0)]
        prev_tiles, prev = None, None
        for u in units[:-1]:
            p2c = []
            if prev:
                # heavy previous PV (qb>=2): push combines later so the PE
                # has drained each group before DVE reaches its reciprocal
                cshift = 0.0
                p2c = std_p2_keys(phase2_chunks(*prev, prev_tiles), cshift,
                                  qb=prev[1])
            prev_tiles = emit_slot(u, p2c)
            prev = u

        # Final slot: weave the previous unit's phase2 with the last unit's
        # OWN phase2 into one 2-bank-rotation-consistent group sequence
        # (pv g_{k+2} emitted after cmb g_k).  Own PV groups key in right
        # after the act blocks of the level they consume, so the drain tail
        # is only the last tiny L1/L2 groups and combines.
        u = units[-1]
        pr = phase2_chunks(prev[0], prev[1], prev_tiles)
        act_blocks = unit_jblocks(u[1])
        own_tiles = {b: pts.tile([128, 2, 512], BF16, tag="pt", name="pt")
                     for b in act_blocks}
        ow = phase2_chunks(u[0], u[1], own_tiles)
        groups = [("P", 0, 0, .04), ("P", 0, 1, .08), ("P", 0, 2, .28),
                  ("P", 1, 0, .33), ("O", 0, 0, .47), ("P", 1, 1, .51),
                  ("O", 1, 0, .55), ("P", 1, 2, .59), ("O", 0, 1, .82),
                  ("O", 1, 1, .85), ("O", 0, 2, .91), ("O", 1, 2, .93)]
        p2c = []
        for k, (src, h, l, key) in enumerate(groups):
            p2 = pr if src == "P" else ow
            p2c.append((key, "pv", p2["pv"](h, l)))
            # cmb(g_k) keyed between g_{k+1} and g_{k+2}
            ck = (groups[k + 2][3] - 0.015) if k + 2 < len(groups) \
                else 0.96 + 0.015 * k
            if src == "O":
                p2c.append((ck, "cmb", p2["cmb"](h, l, l == 2, tail=True)))
            else:
                p2c.append((ck, "cmb", p2["cmb"](h, l, h == 1 and l == 2)))
        emit_slot(u, p2c, own_tiles)
    nc.compile()
    return nc


def _prepare(inputs):
    bf = ml_dtypes.bfloat16
    Q = np.asarray(inputs["Q"], np.float32)
    Ks = [np.asarray(inputs[k], np.float32) for k in ("K0", "K1", "K2")]
    Vs = [np.asarray(inputs[k], np.float32) for k in ("V0", "V1", "V2")]
    level_w = np.asarray(inputs["level_w"], np.float64)
    e = np.exp(level_w - level_w.max())
    w = (e / e.sum()).astype(np.float64)

    # Host-side layout for sharding: per-head transposed Q/K ([64, Tm]) and
    # s-tiled V with a ones column ([128, nj, 65]).  Q carries the 1/8 score
    # scale and the 1/16 poly range-reduction (exact powers of two in bf16);
    # the ACT exp path compensates with activation scale=16.
    QT = np.ascontiguousarray(Q.transpose(0, 1, 3, 2) / 128.0).astype(bf)
    KTs = []
    for Kl in Ks:
        Tm = Kl.shape[1]
        Kh = Kl.reshape(B, Tm, H, DH).transpose(0, 2, 3, 1)  # [B,H,64,Tm]
        KTs.append(np.ascontiguousarray(Kh).astype(bf))
    Vps = []
    for lvl, Vl in enumerate(Vs):
        Tm = Vl.shape[1]
        Vl = Vl * np.float32(w[lvl])   # fold level weight into V (exact in fp32)
        Vh = Vl.reshape(B, Tm, H, DH).transpose(0, 2, 1, 3)  # [B,H,Tm,64]
        vp = np.ones((B, H, Tm // 128, 128, 65), np.float32)
        vp[..., :64] = Vh.reshape(B, H, Tm // 128, 128, DH)
        # -> [B, H, 128(p), nj, 65]
        Vps.append(np.ascontiguousarray(vp.transpose(0, 1, 3, 2, 4)).astype(bf))
    tri1 = (np.arange(128)[:, None] <= np.arange(128)[None, :]).astype(bf)
    tri = np.ascontiguousarray(np.broadcast_to(tri1[:, None, :], (128, 2, 128)))

    in_maps = []
    for core in range(NCORES):
        m = {
            "qt": np.empty((DUOS, 128, T), bf),
            "k0t": np.empty((DUOS, 128, T), bf),
            "k12t": np.empty((DUOS, 128, T1 + T2), bf),
            "vv": np.empty((PAIRS, 128, NJ[0] + NJ[1] + NJ[2], 65), bf),
            "tri": tri,
        }
        for p in range(PAIRS):
            g = PAIRS * core + p
            b, h = divmod(g, H)
            d, half = divmod(p, 2)
            sl = slice(64 * half, 64 * half + 64)
            m["qt"][d, sl] = QT[b, h]
            m["k0t"][d, sl] = KTs[0][b, h]
            m["k12t"][d, sl, 0:T1] = KTs[1][b, h]
            m["k12t"][d, sl, T1:] = KTs[2][b, h]
            m["vv"][p, :, 0:NJ[0]] = Vps[0][b, h]
            m["vv"][p, :, NJ[0]:NJ[0] + NJ[1]] = Vps[1][b, h]
            m["vv"][p, :, NJ[0] + NJ[1]:] = Vps[2][b, h]
        in_maps.append(m)

    return in_maps, w


def kernel(**inputs):
    global LAST_RESULT
    from concourse.bass_utils import run_bass_kernel_spmd

    in_maps, w = _prepare(inputs)
    key = tuple(np.asarray(w, np.float64).tolist())
    if key not in _NC_CACHE:
        _NC_CACHE[key] = _build_nc(w)
    nc = _NC_CACHE[key]

    try:
        res = run_bass_kernel_spmd(nc, in_maps, core_ids=list(range(NCORES)),
                                   trace=TRACE)
    except (ImportError, ModuleNotFoundError):
        # axon build without the NTFF profiling hook — run without trace
        res = run_bass_kernel_spmd(nc, in_maps, core_ids=list(range(NCORES)),
                                   trace=False)
    LAST_RESULT = res

    # out per core: [DUOS, 128(part), 2(half), 16(blk), 64]; pair p = 2d+half,
    # query t = 128*blk + part.
    outs = np.stack([np.asarray(r["out"]) for r in res.results])
    O = outs.transpose(0, 1, 3, 4, 2, 5).reshape(B, H, T, DH)
    return np.ascontiguousarray(O.transpose(0, 2, 1, 3)).reshape(B, T, C).astype(np.float32)

